# revision 8
# baseline (speedup 1.0000x reference)
"""AttentionCTSF Trainium2 kernel — 8-core SPMD Bass implementation.

Pipeline: 1x1x1 conv (W1) -> depthwise (1,3,3) conv -> channel shuffle ->
linear mix along (c1 t) -> qkv -> L2-normalized channel attention over (h w)
-> W_out projection.

Sharding: spatial over h (8 cores x 16 rows, 1-row halo).  The q.k^T Gram
matrices and L2-norm partials are the only cross-core quantities; summed with
one ~420 KB AllReduce.  Everything else is local.  bf16 compute, f32 accum.
"""

import numpy as np
import ml_dtypes

BF16 = ml_dtypes.bfloat16

B, C, T, H, W = 2, 64, 10, 128, 128
C3, C1, C2N = 192, 8, 24
HEADS, CT = 8, 80
NCORES = 8
HL = H // NCORES            # 16 local h rows
HP, WP = HL + 2, W + 2      # 18, 130 padded local dims
SP = HP * WP                # 2340 padded spatial per (b, t)
S = HL * W                  # 2048 interior spatial per (b, t)
NPAIR = B * HEADS           # 16

_cache = {}


def _host_prep(x, W1, Wdw, W_lin, b_lin, temperature, W_out):
    x = np.asarray(x, np.float32)
    xp = np.pad(x, ((0, 0), (0, 0), (0, 0), (1, 1), (1, 1)))
    qk_rows = np.array([c1 * 24 + c2 for c1 in range(C1) for c2 in range(16)])
    v_rows = np.array([c1 * 24 + 16 + c2 for c1 in range(C1) for c2 in range(8)])
    W1 = np.asarray(W1, np.float32)
    W1T_qk = np.ascontiguousarray(W1[qk_rows, :].T).astype(BF16)   # [64,128]
    W1T_v = np.ascontiguousarray(W1[v_rows, :].T).astype(BF16)     # [64,64]
    W_lin = np.asarray(W_lin, np.float32)
    iperm = np.array([c1 * T + t for t in range(T) for c1 in range(C1)])
    W_linT = np.ascontiguousarray(W_lin[:, iperm].T).astype(BF16)  # [80,80]
    wdw = np.asarray(Wdw, np.float32).reshape(C3, 9)
    dwS = np.zeros((80, C2N * 9), np.float32)
    for t in range(T):
        for c1 in range(C1):
            for c2 in range(C2N):
                dwS[t * 8 + c1, c2 * 9:(c2 + 1) * 9] = wdw[c1 * 24 + c2]
    b_lin = np.asarray(b_lin, np.float32)
    biasT = np.ascontiguousarray(np.broadcast_to(b_lin, (128, 80))).astype(BF16)
    b_linP = b_lin.reshape(80, 1).copy()
    W_outT = np.ascontiguousarray(np.asarray(W_out, np.float32).T).astype(BF16)
    temp = np.asarray(temperature, np.float32).reshape(HEADS)
    temp_pair = np.zeros((80, NPAIR), np.float32)
    for b in range(B):
        for hd in range(HEADS):
            temp_pair[:, b * 8 + hd] = temp[hd]
    ones128 = np.ones((128, 1), BF16)
    id80 = np.eye(80, dtype=BF16)
    common = dict(W1T_qk=W1T_qk, W1T_v=W1T_v, W_linT=W_linT, dwS=dwS,
                  biasT=biasT, b_linP=b_linP, W_outT=W_outT,
                  temp_pair=temp_pair, ones128=ones128, id80=id80)
    in_maps = []
    for r in range(NCORES):
        m = dict(common)
        m["xs"] = np.ascontiguousarray(
            xp[:, :, :, r * HL:r * HL + HP, :]).astype(BF16)
        in_maps.append(m)
    return in_maps


def build_program():
    import concourse.bacc as bacc
    import concourse.tile as tile
    from concourse import mybir
    from contextlib import ExitStack

    f32 = mybir.dt.float32
    bf16 = mybir.dt.bfloat16
    Alu = mybir.AluOpType
    Act = mybir.ActivationFunctionType

    nc = bacc.Bacc("TRN2", target_bir_lowering=False, debug=False,
                   num_devices=NCORES)

    xs = nc.dram_tensor("xs", [B, C, T, HP, WP], bf16, kind="ExternalInput")
    ins = {}
    for name, shape, dt in [
        ("W1T_qk", [64, 128], bf16), ("W1T_v", [64, 64], bf16),
        ("W_linT", [80, 80], bf16), ("dwS", [80, C2N * 9], f32),
        ("biasT", [128, 80], bf16), ("b_linP", [80, 1], f32),
        ("W_outT", [64, 64], bf16), ("temp_pair", [80, NPAIR], f32),
        ("ones128", [128, 1], bf16), ("id80", [80, 80], bf16),
    ]:
        ins[name] = nc.dram_tensor(name, shape, dt, kind="ExternalInput")
    y_out = nc.dram_tensor("y_out", [B, C, T, HL, W], f32,
                           kind="ExternalOutput")

    with tile.TileContext(nc) as tc:
        es = ExitStack()
        cpool = es.enter_context(tc.tile_pool(name="consts", bufs=1))
        dram = es.enter_context(tc.tile_pool(name="dram", bufs=1, space="DRAM"))

        cl = {}
        for name, t in ins.items():
            tl = cpool.tile(list(t.shape), t.dtype, tag=name)
            nc.sync.dma_start(tl[:], t.ap())
            cl[name] = tl
        w1qk, w1v, wlin = cl["W1T_qk"], cl["W1T_v"], cl["W_linT"]
        dws, bia, blp = cl["dwS"], cl["biasT"], cl["b_linP"]
        wout, tpa, on1, idt = cl["W_outT"], cl["temp_pair"], cl["ones128"], cl["id80"]

        gpool = es.enter_context(tc.tile_pool(name="gram", bufs=1))
        Gsb = gpool.tile([80, NPAIR, 80], f32, tag="Gsb")
        nsb = gpool.tile([1, NPAIR * 160], f32, tag="nsb")

        v_spill = dram.tile([NPAIR, 80, S], bf16, tag="vspill")
        cc_in = dram.tile([82, 1280], f32, tag="ccin")
        cc_out = dram.tile([82, 1280], f32, tag="ccout")

        taps = [(dh, dw) for dh in range(3) for dw in range(3)]

        def dw_chain(sh, c2loc, c2glob, dwo, eng):
            """9-tap depthwise on padded shuf tile -> interior dwo [80,16,128]."""
            for k, (dh, dw) in enumerate(taps):
                sc = dws[:, c2glob * 9 + k:c2glob * 9 + k + 1]
                src = sh[:, c2loc, :].rearrange(
                    "p (hp wp) -> p hp wp", hp=HP)[:, dh:dh + HL, dw:dw + W]
                if k == 0:
                    eng.tensor_scalar(dwo[:], src, sc, None, Alu.mult)
                else:
                    eng.scalar_tensor_tensor(dwo[:], src, sc, dwo[:],
                                             Alu.mult, Alu.add)

        # ================= phase A+B per batch =================
        for b in range(B):
            shpool = tc.tile_pool(name=f"shuf{b}", bufs=1)
            shp = shpool.__enter__()
            shq = shp.tile([80, 16, SP], bf16, tag="shq")
            shv = shp.tile([80, 8, SP], bf16, tag="shv")

            with tc.tile_pool(name=f"pa{b}", bufs=3) as xa, \
                 tc.tile_pool(name=f"pya{b}", bufs=2) as ya, \
                 tc.tile_pool(name=f"psa{b}", bufs=3, space="PSUM") as psa:
                for t in range(T):
                    xt = xa.tile([64, SP], bf16, tag="xt")
                    nc.sync.dma_start(xt[:], xs.ap()[b, :, t])
                    yqk = ya.tile([128, SP], bf16, tag="yqk")
                    yv = ya.tile([64, SP], bf16, tag="yv")
                    for o, n in ((0, 512), (512, 512), (1024, 512),
                                 (1536, 512), (2048, 292)):
                        pq = psa.tile([128, 512], f32, tag="pq")
                        nc.tensor.matmul(pq[:, :n], w1qk[:], xt[:, o:o + n],
                                         start=True, stop=True)
                        nc.scalar.activation(yqk[:, o:o + n], pq[:, :n],
                                             Act.Copy)
                        pv = psa.tile([128, 512], f32, tag="pq")
                        nc.tensor.matmul(pv[:64, :n], w1v[:], xt[:, o:o + n],
                                         start=True, stop=True)
                        nc.scalar.activation(yv[:, o:o + n], pv[:64, :n],
                                             Act.Copy)
                    nc.sync.dma_start(shq[t * 8:(t + 1) * 8, :, :], yqk[:])
                    nc.sync.dma_start(shv[t * 8:(t + 1) * 8, :, :], yv[:])

            # ---------- phase B ----------
            with tc.tile_pool(name=f"dw{b}", bufs=4) as dwp, \
                 tc.tile_pool(name=f"zq{b}", bufs=4) as zqp, \
                 tc.tile_pool(name=f"vt{b}", bufs=3) as vtp, \
                 tc.tile_pool(name=f"sq{b}", bufs=2) as sqp, \
                 tc.tile_pool(name=f"psd{b}", bufs=2, space="PSUM") as psd, \
                 tc.tile_pool(name=f"psg{b}", bufs=2, space="PSUM") as psg, \
                 tc.tile_pool(name=f"psn{b}", bufs=2, space="PSUM") as psn:
                for hd in range(HEADS):
                    dwq = dwp.tile([80, HL, W], bf16, tag="dwq")
                    dwk = dwp.tile([80, HL, W], bf16, tag="dwq")
                    engq = nc.vector
                    engk = nc.vector
                    dw_chain(shq, hd, hd, dwq, engq)
                    dw_chain(shq, 8 + hd, 8 + hd, dwk, engk)

                    zq2 = sqp.tile([128, 16, 80], bf16, tag="zq2")
                    zk2 = sqp.tile([128, 16, 80], bf16, tag="zq2")
                    gps = psg.tile([80, 80], f32, tag="gps")
                    for ck in range(16):
                        pd = psd.tile([128, 160], f32, tag="pd")
                        dq = dwq[:].rearrange("p h w -> p (h w)").rearrange(
                            "p (c x) -> p c x", c=16)[:, ck, :]
                        dk = dwk[:].rearrange("p h w -> p (h w)").rearrange(
                            "p (c x) -> p c x", c=16)[:, ck, :]
                        nc.tensor.matmul(pd[:, 0:80], dq, wlin[:],
                                         start=True, stop=True)
                        nc.tensor.matmul(pd[:, 80:160], dk, wlin[:],
                                         start=True, stop=True)
                        zq = zqp.tile([128, 80], bf16, tag="zq")
                        zk = zqp.tile([128, 80], bf16, tag="zq")
                        nc.vector.tensor_tensor(zq[:], pd[:, 0:80], bia[:],
                                                Alu.add)
                        nc.vector.tensor_tensor(zk[:], pd[:, 80:160], bia[:],
                                                Alu.add)
                        nc.scalar.activation(zq2[:, ck, :], zq[:], Act.Square)
                        nc.scalar.activation(zk2[:, ck, :], zk[:], Act.Square)
                        nc.tensor.matmul(gps[:], zq[:], zk[:],
                                         start=(ck == 0), stop=(ck == 15))
                    nps = psn.tile([1, 160], f32, tag="nps")
                    for ck in range(16):
                        nc.tensor.matmul(nps[:, 0:80], on1[:], zq2[:, ck, :],
                                         start=(ck == 0), stop=(ck == 15))
                    for ck in range(16):
                        nc.tensor.matmul(nps[:, 80:160], on1[:], zk2[:, ck, :],
                                         start=(ck == 0), stop=(ck == 15))
                    pair = b * 8 + hd
                    nc.scalar.activation(Gsb[:, pair, :], gps[:], Act.Copy)
                    nc.vector.tensor_copy(
                        nsb[:, pair * 160:(pair + 1) * 160], nps[:])

                # ---- v channels ----
                for c2v in range(8):
                    dwv = dwp.tile([80, HL, W], bf16, tag="dwq")
                    engv = nc.vector
                    dw_chain(shv, c2v, 16 + c2v, dwv, engv)
                    vt = vtp.tile([80, S], bf16, tag="vt")
                    dwvf = dwv[:].rearrange("p h w -> p (h w)")
                    for o in (0, 512, 1024, 1536):
                        pv2 = psd.tile([80, 512], f32, tag="pv")
                        nc.tensor.matmul(pv2[:], wlin[:], dwvf[:, o:o + 512],
                                         start=True, stop=True)
                        nc.scalar.activation(vt[:, o:o + 512], pv2[:],
                                             Act.Identity, bias=blp[:],
                                             scale=1.0)
                    nc.sync.dma_start(v_spill[b * 8 + c2v, :, :], vt[:])

            shpool.__exit__(None, None, None)

        # ================= AllReduce =================
        nc.sync.dma_start(cc_in[0:80, :], Gsb[:].rearrange("p a b -> p (a b)"))
        nc.sync.dma_start(cc_in[80:82, :], nsb[:])
        nc.gpsimd.collective_compute(
            "AllReduce", Alu.add,
            replica_groups=[list(range(NCORES))],
            ins=[cc_in.opt()], outs=[cc_out.opt()])

        # ================= softmax + attn@v + W_out =================
        with tc.tile_pool(name="post", bufs=1) as pp, \
             tc.tile_pool(name="att", bufs=4) as ap_, \
             tc.tile_pool(name="vload", bufs=3) as vlp, \
             tc.tile_pool(name="aout", bufs=1) as aop, \
             tc.tile_pool(name="wi", bufs=1) as wip, \
             tc.tile_pool(name="wo", bufs=3) as wop, \
             tc.tile_pool(name="psc", bufs=2, space="PSUM") as psc, \
             tc.tile_pool(name="pso", bufs=2, space="PSUM") as pso:
            Gar = pp.tile([80, NPAIR, 80], f32, tag="Gar")
            nc.sync.dma_start(Gar[:].rearrange("p a b -> p (a b)"),
                              cc_out[0:80, :])
            # norms, two layouts (f32):
            # nq_c [80, 16]: partition=c  (4-byte gather)
            # nk_r [16, 80]: partition=pair
            nq_c = pp.tile([80, NPAIR], f32, tag="nq_c")
            nk1 = pp.tile([1, NPAIR * 80], f32, tag="nk1")
            nflat = cc_out[:].rearrange(
                "p f -> (p f)")[80 * 1280:80 * 1280 + 2560].rearrange(
                "(a q c) -> a q c", a=16, q=2)
            for pair in range(NPAIR):
                nc.sync.dma_start(nq_c[:, pair:pair + 1],
                                  nflat[pair, 0, :].unsqueeze(1))
            nc.sync.dma_start(nk1[:], nflat[:, 1, :])
            # rq = temp / max(sqrt(nq), eps); rk likewise
            rq = pp.tile([80, NPAIR], f32, tag="rq")
            nc.scalar.activation(rq[:], nq_c[:], Act.Sqrt)
            nc.vector.tensor_scalar_max(rq[:], rq[:], 1e-12)
            nc.vector.reciprocal(rq[:], rq[:])
            nc.vector.tensor_tensor(rq[:], rq[:], tpa[:], Alu.mult)
            nc.scalar.activation(nk1[:], nk1[:], Act.Sqrt)
            nc.vector.tensor_scalar_max(nk1[:], nk1[:], 1e-12)
            nc.vector.reciprocal(nk1[:], nk1[:])
            rkb_all = pp.tile([80, NPAIR * 80], f32, tag="rkb_all")
            nc.gpsimd.partition_broadcast(rkb_all[:], nk1[:])

            for b in range(B):
                ao = aop.tile([80, HEADS, S], bf16, tag="ao")
                for hd in range(HEADS):
                    pair = b * 8 + hd
                    lg = ap_.tile([80, 80], f32, tag="lg")
                    nc.vector.scalar_tensor_tensor(
                        lg[:], Gar[:, pair, :], rq[:, pair:pair + 1],
                        rkb_all[:, pair * 80:(pair + 1) * 80],
                        Alu.mult, Alu.mult)
                    mx = ap_.tile([80, 1], f32, tag="mx")
                    nc.vector.tensor_reduce(mx[:], lg[:],
                                            mybir.AxisListType.X, Alu.max)
                    nc.vector.tensor_scalar(lg[:], lg[:], mx[:], None,
                                            Alu.subtract)
                    ex = ap_.tile([80, 80], bf16, tag="ex")
                    sm = ap_.tile([80, 1], f32, tag="mx")
                    nc.scalar.activation(ex[:], lg[:], Act.Exp,
                                         accum_out=sm[:])
                    nc.vector.reciprocal(sm[:], sm[:])
                    at = ap_.tile([80, 80], bf16, tag="ex")
                    nc.vector.tensor_scalar(at[:], ex[:], sm[:], None,
                                            Alu.mult)
                    # transpose attn via PE
                    pt = psc.tile([80, 80], bf16, tag="pt")
                    nc.tensor.transpose(pt[:], at[:], idt[:])
                    atT = ap_.tile([80, 80], bf16, tag="atT")
                    nc.vector.tensor_copy(atT[:], pt[:])
                    # attn @ v
                    vt = vlp.tile([80, S], bf16, tag="vt")
                    nc.sync.dma_start(vt[:], v_spill[pair, :, :])
                    for o in (0, 512, 1024, 1536):
                        pc = psc.tile([80, 512], f32, tag="pc")
                        nc.tensor.matmul(pc[:], atT[:], vt[:, o:o + 512],
                                         start=True, stop=True)
                        nc.scalar.activation(ao[:, hd, o:o + 512], pc[:],
                                             Act.Copy)
                # rearrange attnout [80=(ci,t), hd, S] -> [64=(hd,ci), (t S)]
                wi = wip.tile([64, T, S], bf16, tag="wi")
                for hd in range(HEADS):
                    nc.sync.dma_start(wi[hd * 8:(hd + 1) * 8, :, :],
                                      ao[:, hd, :])
                for t in range(T):
                    ot = wop.tile([64, S], f32, tag="ot")
                    for o in (0, 512, 1024, 1536):
                        po = pso.tile([64, 512], f32, tag="po")
                        nc.tensor.matmul(po[:], wout[:], wi[:, t, o:o + 512],
                                         start=True, stop=True)
                        nc.scalar.activation(ot[:, o:o + 512], po[:], Act.Copy)
                    nc.sync.dma_start(
                        y_out.ap()[b, :, t].rearrange("c h w -> c (h w)"),
                        ot[:])
        es.close()
    nc.compile()
    return nc


def _get_nc():
    if "nc" not in _cache:
        _cache["nc"] = build_program()
    return _cache["nc"]


def kernel(**inputs) -> np.ndarray:
    from concourse.bass_utils import run_bass_kernel_spmd
    nc = _get_nc()
    in_maps = _host_prep(
        inputs["x"], inputs["W1"], inputs["Wdw"], inputs["W_lin"],
        inputs["b_lin"], inputs["temperature"], inputs["W_out"])
    res = run_bass_kernel_spmd(nc, in_maps, core_ids=list(range(NCORES)))
    out = np.empty((B, C, T, H, W), np.float32)
    for r in range(NCORES):
        out[:, :, :, r * HL:(r + 1) * HL, :] = res.results[r]["y_out"]
    return out


# revision 15
# speedup vs baseline: 5304.9819x; 5304.9819x over previous
"""AttentionCTSF Trainium2 kernel — 8-core SPMD Bass implementation.

Pipeline: 1x1x1 conv (W1) -> depthwise (1,3,3) conv -> channel shuffle ->
linear mix along (c1 t) -> qkv -> L2-normalized channel attention over (h w)
-> W_out projection.

Sharding: spatial over h (8 cores x 16 rows, 1-row halo).  The q.k^T Gram
matrices and L2-norm partials are the only cross-core quantities; summed with
one ~420 KB AllReduce.  Everything else is local.  bf16 compute, f32 accum.
"""

import numpy as np
import ml_dtypes

BF16 = ml_dtypes.bfloat16

B, C, T, H, W = 2, 64, 10, 128, 128
C3, C1, C2N = 192, 8, 24
HEADS, CT = 8, 80
NCORES = 8
HL = H // NCORES            # 16 local h rows
HP, WP = HL + 2, W + 2      # 18, 130 padded local dims
SP = HP * WP                # 2340 padded spatial per (b, t)
S = HL * W                  # 2048 interior spatial per (b, t)
NPAIR = B * HEADS           # 16

_cache = {}


def _host_prep(x, W1, Wdw, W_lin, b_lin, temperature, W_out):
    x = np.asarray(x, np.float32)
    xp = np.pad(x, ((0, 0), (0, 0), (0, 0), (1, 1), (1, 1)))
    qk_rows = np.array([c1 * 24 + c2 for c1 in range(C1) for c2 in range(16)])
    v_rows = np.array([c1 * 24 + 16 + c2 for c1 in range(C1) for c2 in range(8)])
    W1 = np.asarray(W1, np.float32)
    W1T_qk = np.ascontiguousarray(W1[qk_rows, :].T).astype(BF16)   # [64,128]
    W1T_v = np.ascontiguousarray(W1[v_rows, :].T).astype(BF16)     # [64,64]
    W_lin = np.asarray(W_lin, np.float32)
    iperm = np.array([c1 * T + t for t in range(T) for c1 in range(C1)])
    W_linT = np.ascontiguousarray(W_lin[:, iperm].T).astype(BF16)  # [80,80]
    wdw = np.asarray(Wdw, np.float32).reshape(C3, 9)
    dwS = np.zeros((80, C2N * 9), np.float32)
    for t in range(T):
        for c1 in range(C1):
            for c2 in range(C2N):
                dwS[t * 8 + c1, c2 * 9:(c2 + 1) * 9] = wdw[c1 * 24 + c2]
    b_lin = np.asarray(b_lin, np.float32)
    bias4 = np.ascontiguousarray(
        np.broadcast_to(np.tile(b_lin, 4), (128, 320))).astype(BF16)
    b_linP = b_lin.reshape(80, 1).copy()
    W_outT = np.ascontiguousarray(np.asarray(W_out, np.float32).T).astype(BF16)
    temp = np.asarray(temperature, np.float32).reshape(HEADS)
    temp_pair = np.zeros((80, NPAIR), np.float32)
    for b in range(B):
        for hd in range(HEADS):
            temp_pair[:, b * 8 + hd] = temp[hd]
    ones128 = np.ones((128, 1), BF16)
    id80 = np.eye(80, dtype=BF16)
    common = dict(W1T_qk=W1T_qk, W1T_v=W1T_v, W_linT=W_linT, dwS=dwS,
                  bias4=bias4, b_linP=b_linP, W_outT=W_outT,
                  temp_pair=temp_pair, ones128=ones128, id80=id80)
    in_maps = []
    for r in range(NCORES):
        m = dict(common)
        m["xs"] = np.ascontiguousarray(
            xp[:, :, :, r * HL:r * HL + HP, :]).astype(BF16)
        in_maps.append(m)
    return in_maps


def build_program(with_ar=True):
    import concourse.bacc as bacc
    import concourse.tile as tile
    from concourse import mybir
    from contextlib import ExitStack

    f32 = mybir.dt.float32
    bf16 = mybir.dt.bfloat16
    Alu = mybir.AluOpType
    Act = mybir.ActivationFunctionType

    nc = bacc.Bacc("TRN2", target_bir_lowering=False, debug=False,
                   num_devices=NCORES if with_ar else 1)

    xs = nc.dram_tensor("xs", [B, C, T, HP, WP], bf16, kind="ExternalInput")
    ins = {}
    for name, shape, dt in [
        ("W1T_qk", [64, 128], bf16), ("W1T_v", [64, 64], bf16),
        ("W_linT", [80, 80], bf16), ("dwS", [80, C2N * 9], f32),
        ("bias4", [128, 320], bf16), ("b_linP", [80, 1], f32),
        ("W_outT", [64, 64], bf16), ("temp_pair", [80, NPAIR], f32),
        ("ones128", [128, 1], bf16), ("id80", [80, 80], bf16),
    ]:
        ins[name] = nc.dram_tensor(name, shape, dt, kind="ExternalInput")
    y_out = nc.dram_tensor("y_out", [B, C, T, HL, W], f32,
                           kind="ExternalOutput")

    with tile.TileContext(nc) as tc:
        es = ExitStack()
        cpool = es.enter_context(tc.tile_pool(name="consts", bufs=1))
        dram = es.enter_context(tc.tile_pool(name="dram", bufs=1, space="DRAM"))

        cl = {}
        for name, t in ins.items():
            tl = cpool.tile(list(t.shape), t.dtype, tag=name)
            nc.sync.dma_start(tl[:], t.ap())
            cl[name] = tl
        w1qk, w1v, wlin = cl["W1T_qk"], cl["W1T_v"], cl["W_linT"]
        dws, bia, blp = cl["dwS"], cl["bias4"], cl["b_linP"]
        wout, tpa, on1, idt = cl["W_outT"], cl["temp_pair"], cl["ones128"], cl["id80"]

        gpool = es.enter_context(tc.tile_pool(name="gram", bufs=1))
        Gsb = gpool.tile([80, NPAIR, 80], f32, tag="Gsb")
        nsb = gpool.tile([1, NPAIR * 160], f32, tag="nsb")

        v_spill = dram.tile([NPAIR, 80, S], bf16, tag="vspill")
        cc_in = dram.tile([82, 1280], f32, tag="ccin")
        cc_out = dram.tile([82, 1280], f32, tag="ccout")

        taps = [(dh, dw) for dh in range(3) for dw in range(3)]

        cidx = [0]

        def dw_chain(sh, c2loc, c2glob, dwo, tmpp):
            """9-tap depthwise on padded shuf tile -> interior dwo [80,16,128].

            Alternates chains between DVE (fused stt) and ACT-scale +
            GpSimd-add to spread the elementwise load across engines."""
            mode = "actpool" if cidx[0] % 5 in (1, 3) else "dve"
            cidx[0] += 1
            for k, (dh, dw) in enumerate(taps):
                sc = dws[:, c2glob * 9 + k:c2glob * 9 + k + 1]
                src = sh[:, c2loc, :].rearrange(
                    "p (hp wp) -> p hp wp", hp=HP)[:, dh:dh + HL, dw:dw + W]
                if mode == "dve":
                    if k == 0:
                        nc.vector.tensor_scalar(dwo[:], src, sc, None, Alu.mult)
                    else:
                        nc.vector.scalar_tensor_tensor(dwo[:], src, sc, dwo[:],
                                                       Alu.mult, Alu.add)
                else:
                    if k == 0:
                        nc.scalar.activation(dwo[:], src, Act.Identity,
                                             bias=0.0, scale=sc)
                    else:
                        tmp = tmpp.tile([80, HL, W], bf16, tag="dwtmp")
                        nc.scalar.activation(tmp[:], src, Act.Identity,
                                             bias=0.0, scale=sc)
                        nc.gpsimd.tensor_tensor(dwo[:], dwo[:], tmp[:], Alu.add)

        # ================= phase A+B per batch =================
        for b in range(B):
            shpool = tc.tile_pool(name=f"shuf{b}", bufs=1)
            shp = shpool.__enter__()
            shq = shp.tile([80, 16, SP], bf16, tag="shq")
            shv = shp.tile([80, 8, SP], bf16, tag="shv")

            with tc.tile_pool(name=f"pa{b}", bufs=3) as xa, \
                 tc.tile_pool(name=f"pya{b}", bufs=2) as ya, \
                 tc.tile_pool(name=f"psa{b}", bufs=3, space="PSUM") as psa:
                for t in range(T):
                    xt = xa.tile([64, SP], bf16, tag="xt")
                    nc.sync.dma_start(xt[:], xs.ap()[b, :, t])
                    yqk = ya.tile([128, SP], bf16, tag="yqk")
                    yv = ya.tile([64, SP], bf16, tag="yv")
                    for o, n in ((0, 512), (512, 512), (1024, 512),
                                 (1536, 512), (2048, 292)):
                        pq = psa.tile([128, 512], f32, tag="pq")
                        nc.tensor.matmul(pq[:, :n], w1qk[:], xt[:, o:o + n],
                                         start=True, stop=True)
                        nc.scalar.activation(yqk[:, o:o + n], pq[:, :n],
                                             Act.Copy)
                        pv = psa.tile([128, 512], f32, tag="pq")
                        nc.tensor.matmul(pv[:64, :n], w1v[:], xt[:, o:o + n],
                                         start=True, stop=True)
                        nc.scalar.activation(yv[:, o:o + n], pv[:64, :n],
                                             Act.Copy)
                    nc.sync.dma_start(shq[t * 8:(t + 1) * 8, :, :], yqk[:])
                    nc.sync.dma_start(shv[t * 8:(t + 1) * 8, :, :], yv[:])

            # ---------- phase B ----------
            with tc.tile_pool(name=f"dw{b}", bufs=5) as dwp, \
                 tc.tile_pool(name=f"zq{b}", bufs=3) as zqp, \
                 tc.tile_pool(name=f"vt{b}", bufs=3) as vtp, \
                 tc.tile_pool(name=f"sq{b}", bufs=2) as sqp, \
                 tc.tile_pool(name=f"psd{b}", bufs=3, space="PSUM") as psd, \
                 tc.tile_pool(name=f"psv{b}", bufs=2, space="PSUM") as psv, \
                 tc.tile_pool(name=f"psg{b}", bufs=2, space="PSUM") as psg, \
                 tc.tile_pool(name=f"psn{b}", bufs=1, space="PSUM") as psn:
                for hd in range(HEADS):
                    dwq = dwp.tile([80, HL, W], bf16, tag="dwq")
                    dwk = dwp.tile([80, HL, W], bf16, tag="dwq")
                    dw_chain(shq, hd, hd, dwq, dwp)
                    dw_chain(shq, 8 + hd, 8 + hd, dwk, dwp)

                    zall = zqp.tile([128, 16, 160], bf16, tag="zall")
                    z2 = sqp.tile([128, 16, 160], bf16, tag="z2")
                    gps = psg.tile([80, 80], f32, tag="gps")
                    dqf = dwq[:].rearrange("p h w -> p (h w)").rearrange(
                        "p (c x) -> p c x", c=16)
                    dkf = dwk[:].rearrange("p h w -> p (h w)").rearrange(
                        "p (c x) -> p c x", c=16)
                    for ck2 in range(8):
                        pd = psd.tile([128, 2, 160], f32, tag="pd")
                        for j in (0, 1):
                            ck = 2 * ck2 + j
                            nc.tensor.matmul(pd[:, j, 0:80], dqf[:, ck, :],
                                             wlin[:], start=True, stop=True)
                            nc.tensor.matmul(pd[:, j, 80:160], dkf[:, ck, :],
                                             wlin[:], start=True, stop=True)
                        nc.vector.tensor_tensor(
                            zall[:, 2 * ck2:2 * ck2 + 2, :], pd[:],
                            bia[:].rearrange("p (a x) -> p a x", a=2),
                            Alu.add)
                        for j in (0, 1):
                            ck = 2 * ck2 + j
                            nc.tensor.matmul(gps[:], zall[:, ck, 0:80],
                                             zall[:, ck, 80:160],
                                             start=(ck == 0), stop=(ck == 15))
                    nc.vector.tensor_tensor(
                        z2[:].rearrange("p a x -> p (a x)"),
                        zall[:].rearrange("p a x -> p (a x)"),
                        zall[:].rearrange("p a x -> p (a x)"), Alu.mult)
                    nps = psn.tile([1, 160], f32, tag="nps")
                    for ck in range(16):
                        nc.tensor.matmul(nps[:, 0:80], on1[:], z2[:, ck, 0:80],
                                         start=(ck == 0), stop=(ck == 15))
                    for ck in range(16):
                        nc.tensor.matmul(nps[:, 80:160], on1[:],
                                         z2[:, ck, 80:160],
                                         start=(ck == 0), stop=(ck == 15))
                    pair = b * 8 + hd
                    nc.scalar.activation(Gsb[:, pair, :], gps[:], Act.Copy)
                    nc.vector.tensor_copy(
                        nsb[:, pair * 160:(pair + 1) * 160], nps[:])

                # ---- v channels ----
                for c2v in range(8):
                    dwv = dwp.tile([80, HL, W], bf16, tag="dwq")
                    dw_chain(shv, c2v, 16 + c2v, dwv, dwp)
                    vt = vtp.tile([80, S], bf16, tag="vt")
                    dwvf = dwv[:].rearrange("p h w -> p (h w)")
                    for o in (0, 512, 1024, 1536):
                        pv2 = psv.tile([80, 512], f32, tag="pv")
                        nc.tensor.matmul(pv2[:], wlin[:], dwvf[:, o:o + 512],
                                         start=True, stop=True)
                        nc.scalar.activation(vt[:, o:o + 512], pv2[:],
                                             Act.Identity, bias=blp[:],
                                             scale=1.0)
                    nc.sync.dma_start(v_spill[b * 8 + c2v, :, :], vt[:])

            shpool.__exit__(None, None, None)

        # ================= AllReduce =================
        nc.sync.dma_start(cc_in[0:80, :], Gsb[:].rearrange("p a b -> p (a b)"))
        nc.sync.dma_start(cc_in[80:82, :], nsb[:])
        if with_ar:
            nc.gpsimd.collective_compute(
                "AllReduce", Alu.add,
                replica_groups=[list(range(NCORES))],
                ins=[cc_in.opt()], outs=[cc_out.opt()])
        else:
            nc.sync.dma_start(cc_out[:], cc_in[:])

        # ================= softmax + attn@v + W_out =================
        with tc.tile_pool(name="post", bufs=1) as pp, \
             tc.tile_pool(name="att", bufs=4) as ap_, \
             tc.tile_pool(name="vload", bufs=3) as vlp, \
             tc.tile_pool(name="aout", bufs=1) as aop, \
             tc.tile_pool(name="wi", bufs=1) as wip, \
             tc.tile_pool(name="wo", bufs=3) as wop, \
             tc.tile_pool(name="psc", bufs=2, space="PSUM") as psc, \
             tc.tile_pool(name="pso", bufs=2, space="PSUM") as pso:
            Gar = pp.tile([80, NPAIR, 80], f32, tag="Gar")
            nc.sync.dma_start(Gar[:].rearrange("p a b -> p (a b)"),
                              cc_out[0:80, :])
            # norms, two layouts (f32):
            # nq_c [80, 16]: partition=c  (4-byte gather)
            # nk_r [16, 80]: partition=pair
            nq_c = pp.tile([80, NPAIR], f32, tag="nq_c")
            nk1 = pp.tile([1, NPAIR * 80], f32, tag="nk1")
            nflat = cc_out[:].rearrange(
                "p f -> (p f)")[80 * 1280:80 * 1280 + 2560].rearrange(
                "(a q c) -> a q c", a=16, q=2)
            for pair in range(NPAIR):
                nc.sync.dma_start(nq_c[:, pair:pair + 1],
                                  nflat[pair, 0, :].unsqueeze(1))
            nc.sync.dma_start(nk1[:], nflat[:, 1, :])
            # rq = temp / max(sqrt(nq), eps); rk likewise
            rq = pp.tile([80, NPAIR], f32, tag="rq")
            nc.scalar.activation(rq[:], nq_c[:], Act.Sqrt)
            nc.vector.tensor_scalar_max(rq[:], rq[:], 1e-12)
            nc.vector.reciprocal(rq[:], rq[:])
            nc.vector.tensor_tensor(rq[:], rq[:], tpa[:], Alu.mult)
            nc.scalar.activation(nk1[:], nk1[:], Act.Sqrt)
            nc.vector.tensor_scalar_max(nk1[:], nk1[:], 1e-12)
            nc.vector.reciprocal(nk1[:], nk1[:])
            rkb_all = pp.tile([80, NPAIR * 80], f32, tag="rkb_all")
            nc.gpsimd.partition_broadcast(rkb_all[:], nk1[:])

            for b in range(B):
                ao = aop.tile([80, HEADS, S], bf16, tag="ao")
                for hd in range(HEADS):
                    pair = b * 8 + hd
                    lg = ap_.tile([80, 80], f32, tag="lg")
                    nc.vector.scalar_tensor_tensor(
                        lg[:], Gar[:, pair, :], rq[:, pair:pair + 1],
                        rkb_all[:, pair * 80:(pair + 1) * 80],
                        Alu.mult, Alu.mult)
                    mx = ap_.tile([80, 1], f32, tag="mx")
                    nc.vector.tensor_reduce(mx[:], lg[:],
                                            mybir.AxisListType.X, Alu.max)
                    nc.vector.tensor_scalar(lg[:], lg[:], mx[:], None,
                                            Alu.subtract)
                    ex = ap_.tile([80, 80], bf16, tag="ex")
                    sm = ap_.tile([80, 1], f32, tag="mx")
                    nc.scalar.activation(ex[:], lg[:], Act.Exp,
                                         accum_out=sm[:])
                    nc.vector.reciprocal(sm[:], sm[:])
                    at = ap_.tile([80, 80], bf16, tag="ex")
                    nc.vector.tensor_scalar(at[:], ex[:], sm[:], None,
                                            Alu.mult)
                    # transpose attn via PE
                    pt = psc.tile([80, 80], bf16, tag="pt")
                    nc.tensor.transpose(pt[:], at[:], idt[:])
                    atT = ap_.tile([80, 80], bf16, tag="atT")
                    nc.vector.tensor_copy(atT[:], pt[:])
                    # attn @ v
                    vt = vlp.tile([80, S], bf16, tag="vt")
                    nc.sync.dma_start(vt[:], v_spill[pair, :, :])
                    for o in (0, 512, 1024, 1536):
                        pc = psc.tile([80, 512], f32, tag="pc")
                        nc.tensor.matmul(pc[:], atT[:], vt[:, o:o + 512],
                                         start=True, stop=True)
                        nc.scalar.activation(ao[:, hd, o:o + 512], pc[:],
                                             Act.Copy)
                # rearrange attnout [80=(ci,t), hd, S] -> [64=(hd,ci), (t S)]
                wi = wip.tile([64, T, S], bf16, tag="wi")
                for hd in range(HEADS):
                    nc.sync.dma_start(wi[hd * 8:(hd + 1) * 8, :, :],
                                      ao[:, hd, :])
                for t in range(T):
                    ot = wop.tile([64, S], f32, tag="ot")
                    for o in (0, 512, 1024, 1536):
                        po = pso.tile([64, 512], f32, tag="po")
                        nc.tensor.matmul(po[:], wout[:], wi[:, t, o:o + 512],
                                         start=True, stop=True)
                        nc.scalar.activation(ot[:, o:o + 512], po[:], Act.Copy)
                    nc.sync.dma_start(
                        y_out.ap()[b, :, t].rearrange("c h w -> c (h w)"),
                        ot[:])
        es.close()
    nc.compile()
    return nc


def _get_nc():
    if "nc" not in _cache:
        _cache["nc"] = build_program()
    return _cache["nc"]


def kernel(**inputs) -> np.ndarray:
    from concourse.bass_utils import run_bass_kernel_spmd
    nc = _get_nc()
    in_maps = _host_prep(
        inputs["x"], inputs["W1"], inputs["Wdw"], inputs["W_lin"],
        inputs["b_lin"], inputs["temperature"], inputs["W_out"])
    res = run_bass_kernel_spmd(nc, in_maps, core_ids=list(range(NCORES)))
    out = np.empty((B, C, T, H, W), np.float32)
    for r in range(NCORES):
        out[:, :, :, r * HL:(r + 1) * HL, :] = res.results[r]["y_out"]
    return out
